# revision 5
# baseline (speedup 1.0000x reference)
"""ChebyKAN Trainium2 kernel.

Reference computation:
    t = tanh(x)                      # x: [8192, 768]
    cheby[b,i,d] = T_d(t[b,i])       # Chebyshev polys, d = 0..8
    out[b,j] = sum_{i,d} cheby[b,i,d] * coefficients[i,j,d]

Strategy (data-parallel over batch across 8 cores):
  - Each core gets a 1024-row batch shard, transposed on host to xt [768, 1024]
    so the contraction dim (in_features) lands on SBUF partitions.
  - out.T[j, b] = sum_k coeffK[k, j] * chebyK[k, b], K = 6*128 i-tiles x 8
    degrees (d=0 contributes a j-constant folded into a bias at PSUM drain).
  - bf16 matmuls, fp32 PSUM accum.  f32r streams the rhs at 4B/partition/cycle
    and measures 272 ns per N=512 matmul under concurrent DMA/DVE SBUF
    traffic; bf16 streams at 2B and issues at ~216 ns (HW-measured).
  - The whole bf16 coefficient array (9.4 MB) is SBUF-resident: one big
    [128, 8*768] tile per i-tile, DMA'd once during the prologue.  Both
    batch-half passes reuse it — no coefficient re-streaming, and the Sync
    queue (which issues DMA descriptors at only ~1.6/us) goes quiet after
    the prologue.  The first i-tile is staged as 8 per-degree slice DMAs
    interleaved with the x tiles so the first matmul isn't gated by a
    1.5 MB transfer (DMA completion latency is ~1.7us on top of transfer).
  - Two passes over batch halves of 512: per pass, all 6 j-tiles accumulate
    in 6 single-bank PSUM tiles over the 48 K-tiles; 576 matmuls total.
  - Chebyshev tiles via product identities: T2=2t^2-1, T3=2tT2-t, T4=2T2^2-1,
    T5=2T2T3-t, T6=2T3^2-1, T7=2T3T4-t, T8=2T4^2-1.  The generator chain
    (t, T2, T3, T4) is kept in f32 — a pure-bf16 recurrence amplifies the
    t-rounding by |T_d'| ~ d^2 near |t|=1 — and each T_d is rounded to bf16
    exactly once for the matmul rhs.  No GpSimd compute: concurrent GpSimd
    elementwise steals DVE ports (~3x DVE slowdown, HW-measured).
  - Deep software pipeline: block b's whole cheby chain (scalar+vector ops)
    is emitted one block ahead of its matmuls, and the tanh pair two blocks
    ahead, so every rhs tile exists a full block before the PE wants it —
    the per-engine queues have ~2x headroom per 10.4us block.
  - The last block of EACH pass runs jt-major with inline staggered PSUM
    drains, so pass-1 matmuls find recycled PSUM banks without stalling.
  - A few dummy matmuls bridge the PE from warm-tile-ready to the first
    real matmul so the HAM throttle window is burnt before real work.
  - Output is written bf16 (halves the drain DMA); host upcasts.
"""

import sys

for _p in ("/opt/trn_rl_repo",):
    if _p not in sys.path:
        sys.path.insert(0, _p)

import ml_dtypes
import numpy as np

import concourse.bass as bass
import concourse.mybir as mybir
import concourse.tile as tile
from concourse import bacc
from concourse import bass_utils
from concourse.tile import TileContext

F32 = mybir.dt.float32
BF16 = mybir.dt.bfloat16
AF = mybir.ActivationFunctionType
OP = mybir.AluOpType

B, I, J, D1 = 8192, 768, 768, 9  # batch, in_features, out_features, degree+1
NCORES = 8
BPC = B // NCORES      # 1024 batch rows per core
IT = I // 128          # 6 i-tiles
KT = IT * 8            # 48 K-tiles (d = 1..8)
JT = J // 128          # 6 j-tiles
HB = 512               # half-batch (matmul N)
NDUMMY = 5             # PE warm-up matmuls

_CACHE = {}


def _build_nc():
    nc = bacc.Bacc("TRN2", target_bir_lowering=False, debug=False,
                   num_devices=NCORES)
    xt = nc.dram_tensor("xt", [I, BPC], F32, kind="ExternalInput").ap()
    # coeff[it, row, dm1*J + j] = coefficients[it*128+row, j, dm1+1]
    coeff = nc.dram_tensor("coeff", [IT, 128, 8 * J], BF16,
                           kind="ExternalInput").ap()
    bias = nc.dram_tensor("bias", [128, JT], F32, kind="ExternalInput").ap()
    out = nc.dram_tensor("out", [J, BPC], BF16, kind="ExternalOutput").ap()

    blocks = [(half, it) for half in range(2) for it in range(IT)]
    NB = len(blocks)

    with TileContext(nc) as tc:
        with (
            tc.tile_pool(name="xtp", bufs=1) as xt_pool,
            tc.tile_pool(name="work", bufs=3) as work,
            tc.tile_pool(name="tanp", bufs=4) as tan_pool,
            tc.tile_pool(name="coeffp", bufs=1) as coeff_pool,
            tc.tile_pool(name="outp", bufs=6) as out_pool,
            tc.tile_pool(name="biasp", bufs=1) as bias_pool,
            tc.tile_pool(name="psum", bufs=8, space="PSUM") as psum_pool,
        ):
            # PE warm-up scratch (zeroed; HAM un-throttles after ~3.4us of
            # sustained matmul activity; the real stream takes over while
            # still inside the cold window).
            warm_f = work.tile([128, HB], F32, name="warm_f", tag="warm_f",
                               bufs=1)
            nc.vector.memset(warm_f, 0.0)
            warm = work.tile([128, HB], BF16, name="warm", tag="warm", bufs=1)
            nc.vector.tensor_copy(warm, warm_f)

            bias_all = bias_pool.tile([128, JT], F32, name="bias_all",
                                      tag="bias_all")

            xt_tiles = [
                xt_pool.tile([128, BPC], F32, name=f"xtt{it}", tag=f"xtt{it}")
                for it in range(IT)
            ]
            ct_tiles = [
                coeff_pool.tile([128, 8 * J], BF16, name=f"ct{it}",
                                tag=f"ct{it}")
                for it in range(IT)
            ]

            def issue_tanh(bi):
                half, it = blocks[bi]
                hs = slice(half * HB, (half + 1) * HB)
                t_b = tan_pool.tile([128, HB], BF16, name="t_b", tag="t_b")
                nc.scalar.activation(t_b, xt_tiles[it][:, hs], AF.Tanh)
                t = tan_pool.tile([128, HB], F32, name="t", tag="t")
                nc.scalar.activation(t, xt_tiles[it][:, hs], AF.Tanh)
                return t, t_b

            def emit_chain(t, t_b):
                """Scalar+vector ops producing the 8 bf16 cheby rhs tiles."""
                # T2 = 2 t^2 - 1
                sq = work.tile([128, HB], F32, name="sq", tag="sq")
                nc.scalar.activation(sq, t, AF.Square)
                T2 = work.tile([128, HB], F32, name="T2", tag="T2")
                nc.vector.tensor_scalar(T2, sq, 2.0, 1.0, OP.mult,
                                        OP.subtract)
                T2_b = work.tile([128, HB], BF16, name="T2_b", tag="T2_b")
                nc.vector.tensor_copy(T2_b, T2)
                # T3 = 2 t T2 - t
                P = work.tile([128, HB], F32, name="P", tag="P")
                nc.vector.tensor_mul(P, t, T2)
                T3 = work.tile([128, HB], F32, name="T3", tag="T3")
                nc.vector.scalar_tensor_tensor(T3, P, 2.0, t, OP.mult,
                                               OP.subtract)
                T3_b = work.tile([128, HB], BF16, name="T3_b", tag="T3_b")
                nc.scalar.activation(T3_b, T3, AF.Identity)
                # T4 = 2 T2^2 - 1
                sq = work.tile([128, HB], F32, name="sq", tag="sq")
                nc.scalar.activation(sq, T2, AF.Square)
                T4 = work.tile([128, HB], F32, name="T4", tag="T4")
                nc.vector.tensor_scalar(T4, sq, 2.0, 1.0, OP.mult,
                                        OP.subtract)
                T4_b = work.tile([128, HB], BF16, name="T4_b", tag="T4_b")
                nc.vector.tensor_copy(T4_b, T4)
                # T5 = 2 T2 T3 - t
                P = work.tile([128, HB], F32, name="P", tag="P")
                nc.vector.tensor_mul(P, T2, T3)
                T5_b = work.tile([128, HB], BF16, name="T5_b", tag="T5_b")
                nc.vector.scalar_tensor_tensor(T5_b, P, 2.0, t, OP.mult,
                                               OP.subtract)
                # T6 = 2 T3^2 - 1
                sq = work.tile([128, HB], F32, name="sq", tag="sq")
                nc.scalar.activation(sq, T3, AF.Square)
                T6_b = work.tile([128, HB], BF16, name="T6_b", tag="T6_b")
                nc.vector.tensor_scalar(T6_b, sq, 2.0, 1.0, OP.mult,
                                        OP.subtract)
                # T7 = 2 T3 T4 - t
                P = work.tile([128, HB], F32, name="P", tag="P")
                nc.vector.tensor_mul(P, T3, T4)
                T7_b = work.tile([128, HB], BF16, name="T7_b", tag="T7_b")
                nc.vector.scalar_tensor_tensor(T7_b, P, 2.0, t, OP.mult,
                                               OP.subtract)
                # T8 = 2 T4^2 - 1
                sq = work.tile([128, HB], F32, name="sq", tag="sq")
                nc.scalar.activation(sq, T4, AF.Square)
                T8_b = work.tile([128, HB], BF16, name="T8_b", tag="T8_b")
                nc.vector.tensor_scalar(T8_b, sq, 2.0, 1.0, OP.mult,
                                        OP.subtract)
                return (t_b, T2_b, T3_b, T4_b, T5_b, T6_b, T7_b, T8_b)

            # Prologue DMA order on the Sync queue == rough transfer/land
            # order.  The first tanh needs xt0's first half; the first
            # matmul needs coeff it=0/d=1; everything else follows at the
            # rate blocks consume it.
            nc.sync.dma_start(xt_tiles[0][:, :HB], xt[0:128, :HB])
            nc.sync.dma_start(ct_tiles[0][:, 0:J], coeff[0][:, 0:J])
            nc.sync.dma_start(ct_tiles[0][:, J:2 * J], coeff[0][:, J:2 * J])
            nc.sync.dma_start(xt_tiles[0][:, HB:], xt[0:128, HB:])
            for dm1 in range(2, 8):
                nc.sync.dma_start(ct_tiles[0][:, dm1 * J:(dm1 + 1) * J],
                                  coeff[0][:, dm1 * J:(dm1 + 1) * J])
            nc.sync.dma_start(xt_tiles[1], xt[128:256, :])
            nc.sync.dma_start(ct_tiles[1], coeff[1])
            nc.sync.dma_start(bias_all, bias)
            for it in range(2, IT):
                nc.sync.dma_start(xt_tiles[it], xt[it * 128:(it + 1) * 128, :])
                nc.sync.dma_start(ct_tiles[it], coeff[it])

            # Software-pipeline prologue: tanh(0), chain(0), tanh(1).
            t_pipe = [None] * NB
            chain_pipe = [None] * NB
            t_pipe[0] = issue_tanh(0)
            chain_pipe[0] = emit_chain(*t_pipe[0])
            t_pipe[1] = issue_tanh(1)

            ps = None
            for bi, (half, it) in enumerate(blocks):
                hs = slice(half * HB, (half + 1) * HB)
                ctt = ct_tiles[it]
                if it == 0:
                    ps = [psum_pool.tile([128, HB], F32, name="ps", tag="ps")
                          for _ in range(JT)]
                if bi == 0:
                    # dummy matmuls bridge PE from warm-tile-ready to the
                    # first real matmul; overwritten by the real k==0 matmul
                    # (start=True clears has_written)
                    for _ in range(NDUMMY):
                        nc.tensor.matmul(ps[0], lhsT=warm[:, :128], rhs=warm,
                                         start=True, stop=True)

                # Produce block bi+1's chain and block bi+2's tanh now, so
                # every rhs tile exists a block before the PE wants it.
                if bi + 1 < NB:
                    chain_pipe[bi + 1] = emit_chain(*t_pipe[bi + 1])
                if bi + 2 < NB:
                    t_pipe[bi + 2] = issue_tanh(bi + 2)

                Ts = chain_pipe[bi]
                if it == IT - 1:
                    # Last block of each pass: jt-major so each j-tile's
                    # accumulation finishes staggered and its PSUM drain
                    # (copy + store) pipelines behind the remaining matmuls;
                    # the next pass finds recycled banks without stalling.
                    for jt in range(JT):
                        for dm1, Td in enumerate(Ts):
                            k = it * 8 + dm1
                            nc.tensor.matmul(
                                ps[jt],
                                lhsT=ctt[:, dm1 * J + jt * 128:
                                         dm1 * J + (jt + 1) * 128],
                                rhs=Td,
                                start=(k == 0),
                                stop=(k == KT - 1),
                            )
                        ob = out_pool.tile([128, HB], BF16, name="ob",
                                           tag="ob")
                        if jt % 2 == 0:
                            nc.scalar.activation(
                                ob, ps[jt], AF.Identity,
                                bias=bias_all[:, jt:jt + 1])
                            nc.scalar.dma_start(
                                out[jt * 128:(jt + 1) * 128, hs], ob)
                        else:
                            nc.vector.tensor_scalar_add(
                                ob, ps[jt], bias_all[:, jt:jt + 1])
                            nc.sync.dma_start(
                                out[jt * 128:(jt + 1) * 128, hs], ob)
                else:
                    for dm1, Td in enumerate(Ts):
                        k = it * 8 + dm1
                        for jt in range(JT):
                            nc.tensor.matmul(
                                ps[jt],
                                lhsT=ctt[:, dm1 * J + jt * 128:
                                         dm1 * J + (jt + 1) * 128],
                                rhs=Td,
                                start=(k == 0),
                                stop=(k == KT - 1),
                            )

    nc.compile()
    return nc


def _get_nc():
    if "nc" not in _CACHE:
        _CACHE["nc"] = _build_nc()
    return _CACHE["nc"]


def _prep_inputs(x, coefficients):
    x = np.asarray(x, dtype=np.float32)
    coefficients = np.asarray(coefficients, dtype=np.float32)
    xt_full = np.ascontiguousarray(x.T)  # [768, 8192]

    # coeff[it, row, dm1*J + j] = coefficients[it*128+row, j, dm1+1]
    cr = coefficients.reshape(IT, 128, J, D1)
    arr = np.transpose(cr[:, :, :, 1:], (0, 1, 3, 2))  # [6, 128, 8, 768]
    coeff_in = np.ascontiguousarray(
        arr.reshape(IT, 128, 8 * J).astype(ml_dtypes.bfloat16))

    bias_in = np.ascontiguousarray(
        coefficients[:, :, 0].sum(axis=0).astype(np.float32).reshape(JT, 128).T
    )

    in_maps = []
    for c in range(NCORES):
        xt_c = np.ascontiguousarray(xt_full[:, c * BPC:(c + 1) * BPC])
        in_maps.append({"xt": xt_c, "coeff": coeff_in, "bias": bias_in})
    return in_maps


def _run(x, coefficients, trace=False, **run_kwargs):
    nc = _get_nc()
    in_maps = _prep_inputs(x, coefficients)
    res = bass_utils.run_bass_kernel_spmd(
        nc, in_maps, core_ids=list(range(NCORES)), trace=trace, **run_kwargs
    )
    out_full = np.empty((B, J), dtype=np.float32)
    for c in range(NCORES):
        out_full[c * BPC:(c + 1) * BPC, :] = \
            res.results[c]["out"].T.astype(np.float32)
    return out_full, res


def kernel(x, coefficients):
    out, _ = _run(x, coefficients, trace=False)
    return out


if __name__ == "__main__":
    rng = np.random.default_rng(0)
    x = rng.standard_normal((B, I), dtype=np.float32)
    std = 1.0 / (I * D1)
    coefficients = (std * rng.standard_normal((I, J, D1))).astype(np.float32)
    out = kernel(x, coefficients)
    print("out", out.shape, out.dtype, float(np.abs(out).mean()))
